# revision 10
# baseline (speedup 1.0000x reference)
"""Trainium2 Bass kernel for the CPC loss (nn_CPC_292057776614), v10.

v10 engine-rebalance after v9 trace analysis (fp8-byte output runs at
only ~0.5 elem/ns on DVE/ACT, GpSimd chains in the stage-A pool scope
serialize the A->B transition behind a ucode DRAIN, and the per-tile
[128,2016] reduce_max makes DVE the stage-B critical path):
  - predT exists ONLY in fp8 (predT8).  The positive logit d0 is no
    longer a DVE elementwise chain: each p-tile computes a 128x128
    Gram matmul predT8^T @ encTtgt8 on the PE (5 DoubleRow k-pairs)
    and extracts its diagonal with ONE fused DVE op
    (scalar_tensor_tensor: out=(gram*1)*I, accum_out=row-sum = diag).
    Host-simulated numerics: loss rel ~1.6e-3, corr rel ~2.1e-3.
  - Stage-A PSUM evacuation splits between ACT (512-col group,
    activation w/ bias+1/16 scale) and DVE (160+480-col groups,
    tensor_scalar mult+add) so neither engine paces the PE.
  - Stage-B mask-adds: dense chunks on DVE, cold chunks on GpSimd
    (emitted per-tile in the stage-B scope).
  - 512-wide DoubleRow chunks; RD=1280 dense / GT=736 cold.

Sharding (8 cores, data-parallel over predictions, step-sharded
weights) and the popularity split are unchanged from v7; all big
tensors are TRN e4m3 (W pre-scaled by 16, undone at evac), f32 PSUM.
"""

import numpy as np
import ml_dtypes

import concourse.bass as bass
import concourse.mybir as mybir
import concourse.tile as tile
from concourse import bacc
from concourse.bass_utils import run_bass_kernel_spmd

BF16 = mybir.dt.bfloat16
F32 = mybir.dt.float32
FP8 = mybir.dt.float8e4
E4 = ml_dtypes.float8_e4m3

# Problem constants (hardcoded; kernel.py must be self-contained).
B, G, D, S, NEG = 64, 7, 1280, 5, 16
CELLS = G * G            # 49
R = B * CELLS            # 3136 rows in ctx/enc
K17 = NEG + 1            # 17 candidates per prediction
STEP_LENS = [B * (G - 1 - s) * G for s in range(S)]     # [2688,2240,1792,1344,896]
P_TOTAL = sum(STEP_LENS)                                # 8960
N_CORES = 8
PC = 1120                # predictions per core
NT = 9                   # p-tiles of 128
PP = NT * 128            # 1152 padded
KD = D // 128            # 10 dout/din chunks
NKP = KD // 2            # 5 DoubleRow k-pairs
NEGINF = -240.0          # fp8 mask sentinel (scores are |s| < ~210)
WSCALE = 16.0            # power-of-two weight pre-scale (host), undone at evac

RD = 1280                # dense width
GT = 736                 # per-tile cold slots (max cold on this data = 724)
MW = RD + GT             # 2016 masked width per tile
DCH = 512                # max chunk width
BCHUNKS = [(0, 512, False), (512, 1024, False), (1024, 1280, False),
           (0, 512, True), (512, 736, True)]
NBC = len(BCHUNKS)       # 5
GPS_COLD = False         # GpSimd cannot read PSUM (walrus rejects it)

SEGW = (672, 448)
ASSIGN = [
    ((0, 0),    (1, 1344)),
    ((0, 672),  (1, 1792)),
    ((0, 1344), (2, 0)),
    ((0, 2016), (2, 448)),
    ((1, 0),    (2, 896)),
    ((1, 672),  (2, 1344)),
    ((3, 0),    (4, 0)),
    ((3, 672),  (4, 448)),
]
COLGROUPS = [(0, 512, 0), (512, 672, 0), (672, 1152, 1)]

_CACHE = {}
DR = mybir.MatmulPerfMode.DoubleRow


def _stage_a(nc, tc, ctxT_d, Wg_d, bias_d, encTtgt_d, encT_d, ecold_d, cmask_d,
             vmask_d, sbufs, ctxT_sb, encTtgt_sb, prefetch, vmask_sb):
    """Stage A: predT8 = fp8(ctx @ W^T / 16 + bias), DoubleRow matmuls."""
    IDENT = mybir.ActivationFunctionType.Identity
    AL = mybir.AluOpType
    bias_sb = sbufs["bias"]
    predT8_sb = sbufs["predT8"]
    encT_sb = sbufs["encT"]
    with (
        tc.tile_pool(name="wpool", bufs=4) as wpool,
        tc.tile_pool(name="psa", bufs=4, space="PSUM") as psa,
    ):
        # Weights + ctxT first in the HWDGE FIFO (stage A essentials,
        # ~4.8 MB vs ~25 us of stage-A PE time); everything else after.
        # NOTE: every dma_start must be EMITTED before its first reader
        # (the tile framework orders a later-emitted write after earlier
        # readers, so a late DMA means the reader sees uninit SBUF).
        wtiles = []
        for m in range(2):
            wab = wpool.tile([128, 2, KD, 128], FP8, tag="w")
            if m == 0:
                nc.sync.dma_start(
                    out=wab[:, :, 0:2, :], in_=Wg_d.ap()[m][:, :, 0:2, :]
                )
                nc.sync.dma_start(
                    out=ctxT_sb[:, 0:2, :], in_=ctxT_d.ap()[:, 0:2, :]
                )
                nc.sync.dma_start(
                    out=wab[:, :, 2:KD, :], in_=Wg_d.ap()[m][:, :, 2:KD, :]
                )
                nc.sync.dma_start(
                    out=ctxT_sb[:, 2:4, :], in_=ctxT_d.ap()[:, 2:4, :]
                )
            else:
                nc.sync.dma_start(out=wab[:], in_=Wg_d.ap()[m])
                nc.sync.dma_start(
                    out=ctxT_sb[:, 4:6, :], in_=ctxT_d.ap()[:, 4:6, :]
                )
            wtiles.append(wab)
        for k in range(6, KD, 2):
            nc.sync.dma_start(
                out=ctxT_sb[:, k : k + 2, :], in_=ctxT_d.ap()[:, k : k + 2, :]
            )
        nc.sync.dma_start(out=bias_sb[:], in_=bias_d.ap())

        for m in range(KD):
            if m < 2:
                wab = wtiles[m]
            else:
                wab = wpool.tile([128, 2, KD, 128], FP8, tag="w")
                nc.sync.dma_start(out=wab[:], in_=Wg_d.ap()[m])
            # post-weight streams: dense encT, tile-0 cold data, vmask;
            # encTtgt defers into stage B (grams are shifted one tile)
            if m in (8, 9):
                c0_, c1_ = {8: (0, 640), 9: (640, RD)}[m]
                cs = slice(c0_, c1_)
                nc.sync.dma_start(out=encT_sb[:, :, cs], in_=encT_d.ap()[:, :, cs])
            if m == 9:
                ec0, cm0 = prefetch[0]
                nc.sync.dma_start(out=ec0[:], in_=ecold_d.ap()[0])
                nc.sync.dma_start(out=cm0[:], in_=cmask_d.ap()[0:128, :])
                nc.sync.dma_start(out=vmask_sb[:], in_=vmask_d.ap())
                nc.sync.dma_start(
                    out=encTtgt_sb[:, 0:4, :], in_=encTtgt_d.ap()[:, 0:4, :]
                )

            for c0, c1, seg in COLGROUPS:
                ps = psa.tile([128, DCH], F32, tag="ps")
                for kp in range(NKP):
                    nc.tensor.matmul(
                        ps[:, : c1 - c0],
                        lhsT=wab[:, seg, 2 * kp : 2 * kp + 2, :],
                        rhs=ctxT_sb[:, 2 * kp : 2 * kp + 2, c0:c1],
                        start=(kp == 0),
                        stop=(kp == NKP - 1),
                        perf_mode=DR,
                    )
                bcol = bias_sb[:, seg * KD + m : seg * KD + m + 1]
                if c0 == 0:
                    # ACT evac for the wide group
                    nc.scalar.activation(
                        predT8_sb[:, m, c0:c1],
                        ps[:, : c1 - c0],
                        IDENT,
                        bias=bcol,
                        scale=1.0 / WSCALE,
                    )
                else:
                    # DVE evac for the rest: (psum * 1/16) + bias
                    nc.any.tensor_scalar(
                        out=predT8_sb[:, m, c0:c1],
                        in0=ps[:, : c1 - c0],
                        scalar1=1.0 / WSCALE,
                        scalar2=bcol,
                        op0=AL.mult,
                        op1=AL.add,
                    )


def _build():
    if "nc" in _CACHE:
        return _CACHE["nc"]

    nc = bacc.Bacc("TRN2", target_bir_lowering=False, debug=False)

    ctxT_d = nc.dram_tensor("ctxT", [128, KD, PP], FP8, kind="ExternalInput")
    Wg_d = nc.dram_tensor("Wg", [KD, 128, 2, KD, 128], FP8, kind="ExternalInput")
    bias_d = nc.dram_tensor("biasT", [128, 2 * KD], F32, kind="ExternalInput")
    encT_d = nc.dram_tensor("encT", [128, KD, RD], FP8, kind="ExternalInput")
    ecold_d = nc.dram_tensor("ecold", [NT, 128, KD, GT], FP8, kind="ExternalInput")
    encTtgt_d = nc.dram_tensor("encTtgt", [128, KD, PP], FP8, kind="ExternalInput")
    cmask_d = nc.dram_tensor("cmask", [PP, MW], FP8, kind="ExternalInput")
    vmask_d = nc.dram_tensor("vmask", [128, NT], F32, kind="ExternalInput")
    ident_d = nc.dram_tensor("ident", [128, 128], BF16, kind="ExternalInput")
    out_d = nc.dram_tensor("out", [1, 2], F32, kind="ExternalOutput")

    EXP = mybir.ActivationFunctionType.Exp
    LN = mybir.ActivationFunctionType.Ln
    X = mybir.AxisListType.X
    AL = mybir.AluOpType

    with tile.TileContext(nc) as tc:
        with (
            tc.tile_pool(name="const", bufs=1) as const,
            tc.tile_pool(name="spool", bufs=1) as spool,
            tc.tile_pool(name="ctxp", bufs=1) as ctxp,
            tc.tile_pool(name="psf", bufs=1, space="PSUM") as psf_pool,
        ):
            sbufs = {
                "bias": const.tile([128, 2 * KD], F32, name="bias_sb"),
                "predT8": const.tile([128, KD, PP], FP8, name="predT8_sb"),
                "encT": const.tile([128, KD, RD], FP8, name="encT_sb"),
                "dots0": const.tile([128, NT], F32, name="dots0_sb"),
                "ones": const.tile([128, 1], F32, name="ones"),
            }
            ctxT_sb = ctxp.tile([128, KD, PP], FP8)
            encTtgt_sb = ctxp.tile([128, KD, PP], FP8)
            ident_sb = const.tile([128, 128], BF16)
            vmask_sb = const.tile([128, NT], F32)
            nmax_sb = const.tile([128, NT], F32)
            sume_sb = const.tile([128, NT], F32)
            nc.vector.memset(sbufs["ones"][:], 1.0)
            nc.sync.dma_start(out=ident_sb[:], in_=ident_d.ap())
            dots0_sb = sbufs["dots0"]
            predT8_sb = sbufs["predT8"]
            encT_sb = sbufs["encT"]

            # ---- finale: stable loss/corr combine, mask, reduce ----
            # phase1 (per column group, EXP-only so the ACT table stays
            # resident); phase2 (all 9 cols at once, single LN table
            # load in the tail)
            res = spool.tile([128, 2 * NT], F32)
            tmax_all = const.tile([128, NT], F32)
            m2_all = const.tile([128, NT], F32)
            st2_all = const.tile([128, NT], F32)

            def _finale_phase1(cs, n):
                tmax = tmax_all[:, cs]
                nc.vector.tensor_scalar_mul(tmax, nmax_sb[:, cs], -1.0)
                m2 = m2_all[:, cs]
                nc.vector.tensor_tensor(
                    out=m2, in0=tmax, in1=dots0_sb[:, cs],
                    op=mybir.AluOpType.max,
                )
                ea_arg = spool.tile([128, n], F32, name=f"eaa{n}")
                nc.vector.tensor_sub(ea_arg[:], tmax, m2)
                eb_arg = spool.tile([128, n], F32, name=f"eba{n}")
                nc.vector.tensor_sub(eb_arg[:], dots0_sb[:, cs], m2)
                ea = spool.tile([128, n], F32, name=f"ea{n}")
                nc.scalar.activation(ea[:], ea_arg[:], EXP, bias=0.0, scale=1.0)
                eb = spool.tile([128, n], F32, name=f"eb{n}")
                nc.scalar.activation(eb[:], eb_arg[:], EXP, bias=0.0, scale=1.0)
                st1 = spool.tile([128, n], F32, name=f"st1{n}")
                nc.vector.tensor_mul(st1[:], sume_sb[:, cs], ea[:])
                nc.vector.tensor_add(st2_all[:, cs], st1[:], eb[:])

            def _finale_phase2():
                lnt = spool.tile([128, NT], F32, name="lnt")
                nc.scalar.activation(lnt[:], st2_all[:], LN, bias=0.0, scale=1.0)
                l0 = spool.tile([128, NT], F32, name="l0")
                nc.vector.tensor_add(l0[:], lnt[:], m2_all[:])
                lossp = spool.tile([128, NT], F32, name="lossp")
                nc.vector.tensor_sub(lossp[:], l0[:], dots0_sb[:])
                corrp = spool.tile([128, NT], F32, name="corrp")
                nc.vector.tensor_tensor(
                    out=corrp[:], in0=dots0_sb[:], in1=tmax_all[:],
                    op=mybir.AluOpType.is_ge,
                )
                nc.vector.tensor_mul(res[:, 0:NT], lossp[:], vmask_sb[:])
                nc.vector.tensor_mul(res[:, NT : 2 * NT], corrp[:], vmask_sb[:])

            with (
                tc.tile_pool(name="ecp", bufs=3) as ecp_pool,
                tc.tile_pool(name="cmp", bufs=3) as cmp_pool,
            ):
                ec0 = ecp_pool.tile([128, KD, GT], FP8, tag="ec")
                ec1 = ecp_pool.tile([128, KD, GT], FP8, tag="ec")
                cm0 = cmp_pool.tile([128, MW], FP8, tag="cm")
                cm1 = cmp_pool.tile([128, MW], FP8, tag="cm")

                _stage_a(
                    nc, tc, ctxT_d, Wg_d, bias_d, encTtgt_d, encT_d,
                    ecold_d, cmask_d, vmask_d, sbufs, ctxT_sb, encTtgt_sb,
                    prefetch=[(ec0, cm0), (ec1, cm1)], vmask_sb=vmask_sb,
                )

                # ---- stage B: gram d0 + dense/cold scores, masked stats ----
                with (
                    tc.tile_pool(name="mpool", bufs=2) as mpool,
                    tc.tile_pool(name="trash", bufs=1) as trashpool,
                    tc.tile_pool(name="psb", bufs=5, space="PSUM") as psb,
                    tc.tile_pool(name="psd", bufs=2, space="PSUM") as psd,
                ):
                    for t in range(NT):
                        if t == 0:
                            nc.sync.dma_start(out=ec1[:], in_=ecold_d.ap()[1])
                            nc.sync.dma_start(
                                out=cm1[:], in_=cmask_d.ap()[128:256, :]
                            )
                            nc.sync.dma_start(
                                out=encTtgt_sb[:, 4:KD, :],
                                in_=encTtgt_d.ap()[:, 4:KD, :],
                            )
                        rows = slice(t * 128, (t + 1) * 128)
                        if t == 0:
                            ec, cm = ec0, cm0
                        elif t == 1:
                            ec, cm = ec1, cm1
                        else:
                            ec = ecp_pool.tile([128, KD, GT], FP8, tag="ec")
                            nc.sync.dma_start(out=ec[:], in_=ecold_d.ap()[t])
                            cm = cmp_pool.tile([128, MW], FP8, tag="cm")
                            nc.sync.dma_start(out=cm[:], in_=cmask_d.ap()[rows, :])
                        def _gram(tt):
                            # d0 tile tt: diag(predT8^T @ encTtgt8)
                            # (full-bank tile: a 512B gram sharing a 2KB
                            # PSUM bank with the next gram races with its
                            # start=True zero-region)
                            grows = slice(tt * 128, (tt + 1) * 128)
                            pg_full = psd.tile([128, DCH], F32, tag="pg")
                            pg = pg_full[:, 0:128]
                            for kp in range(NKP):
                                nc.tensor.matmul(
                                    pg[:],
                                    lhsT=predT8_sb[:, 2 * kp : 2 * kp + 2, grows],
                                    rhs=encTtgt_sb[:, 2 * kp : 2 * kp + 2, grows],
                                    start=(kp == 0),
                                    stop=(kp == NKP - 1),
                                    perf_mode=DR,
                                )
                            gdump = spool.tile(
                                [128, 128], F32, tag="gd", name="gdump"
                            )
                            nc.vector.scalar_tensor_tensor(
                                out=gdump[:], in0=pg[:], scalar=1.0,
                                in1=ident_sb[:], op0=AL.mult, op1=AL.mult,
                                accum_out=dots0_sb[:, tt : tt + 1],
                            )

                        if t >= 1:
                            _gram(t - 1)
                        masked = mpool.tile([128, MW], F32, tag="mk")
                        for ci, (c0, c1, is_cold) in enumerate(BCHUNKS):
                            src = ec if is_cold else encT_sb
                            mcols = (
                                slice(RD + c0, RD + c1)
                                if is_cold
                                else slice(c0, c1)
                            )
                            pb = psb.tile([128, DCH], F32, tag="pb")
                            for kp in range(NKP):
                                nc.tensor.matmul(
                                    pb[:, : c1 - c0],
                                    lhsT=predT8_sb[:, 2 * kp : 2 * kp + 2, rows],
                                    rhs=src[:, 2 * kp : 2 * kp + 2, c0:c1],
                                    start=(kp == 0),
                                    stop=(kp == NKP - 1),
                                    perf_mode=DR,
                                )
                            eng = nc.gpsimd if (GPS_COLD and is_cold) else nc.vector
                            eng.tensor_add(
                                masked[:, mcols], pb[:, : c1 - c0], cm[:, mcols]
                            )

                        if t == NT - 1:
                            _gram(NT - 1)
                        nc.vector.reduce_max(
                            nmax_sb[:, t : t + 1],
                            masked[:],
                            axis=X,
                            negate=True,
                        )
                        if t == NT - 1:
                            # overlaps the last exp on ACT
                            _finale_phase1(slice(0, NT - 1), NT - 1)
                        Et = trashpool.tile([128, MW], BF16, tag="E")
                        nc.scalar.activation(
                            Et[:],
                            masked[:],
                            EXP,
                            bias=nmax_sb[:, t : t + 1],
                            scale=1.0,
                            accum_out=sume_sb[:, t : t + 1],
                        )

            _finale_phase1(slice(NT - 1, NT), 1)
            _finale_phase2()

            fin = spool.tile([128, 2], F32)
            nc.vector.reduce_sum(fin[:, 0:1], res[:, 0:NT], axis=X)
            nc.vector.reduce_sum(fin[:, 1:2], res[:, NT : 2 * NT], axis=X)
            pf = psf_pool.tile([1, 2], F32)
            nc.tensor.matmul(
                pf[:], lhsT=sbufs["ones"][:], rhs=fin[:], start=True, stop=True
            )
            out_sb = const.tile([1, 2], F32)
            nc.vector.tensor_copy(out_sb[:], pf[:])
            nc.sync.dma_start(out=out_d.ap(), in_=out_sb[:])

    nc.compile()
    _CACHE["nc"] = nc
    return nc


def _to_partfirst(a2d):
    """[D, N] -> [128, KD, N] with global dim j = k*128 + part."""
    Dd, N = a2d.shape
    return np.ascontiguousarray(a2d.reshape(KD, 128, N).transpose(1, 0, 2))


def _prep_in_maps(contexts, encodings, Wk_w, Wk_b, ctx_idx, cand_idx):
    ctx8 = np.asarray(contexts, dtype=np.float32).reshape(R, D).astype(E4)
    enc8 = np.asarray(encodings, dtype=np.float32).reshape(R, D).astype(E4)
    Wk_w = np.asarray(Wk_w, dtype=np.float32)
    Wk_b = np.asarray(Wk_b, dtype=np.float32)
    ctx_idx = np.asarray(ctx_idx, dtype=np.int32)
    cand_idx = np.asarray(cand_idx, dtype=np.int32)

    offs = np.concatenate([[0], np.cumsum(STEP_LENS)]).astype(np.int64)

    vmask = np.ascontiguousarray(
        (np.arange(PP) < PC).astype(np.float32).reshape(NT, 128).T
    )
    ident = np.eye(128, dtype=ml_dtypes.bfloat16)

    # per-step weight granules [KD_m, 128, KD_k, 128], pre-scaled by 16
    wgr = {}
    for s in range(S):
        WT = (Wk_w[s].T * WSCALE).astype(E4)                        # [j, i]
        wgr[s] = np.ascontiguousarray(
            WT.reshape(KD, 128, KD, 128).transpose(2, 1, 0, 3)
        )
    bcol = {}
    for s in range(S):
        bcol[s] = np.ascontiguousarray(Wk_b[s].reshape(KD, 128).T)  # [128, KD]

    in_maps = []
    for c in range(N_CORES):
        (sA, oA), (sB, oB) = ASSIGN[c]
        idx = np.concatenate(
            [
                np.arange(offs[sA] + oA, offs[sA] + oA + SEGW[0]),
                np.arange(offs[sB] + oB, offs[sB] + oB + SEGW[1]),
            ]
        )
        ci = ctx_idx[idx]                                           # [1120]
        ki = cand_idx[idx].astype(np.int64)                         # [1120, 17]

        ctx_g = np.zeros((PP, D), E4)
        ctx_g[:PC] = ctx8[ci]
        ctxT = _to_partfirst(ctx_g.T)

        tgt_g = np.zeros((PP, D), E4)
        tgt_g[:PC] = enc8[ki[:, 0]]
        encTtgt = _to_partfirst(tgt_g.T)

        Wg = np.ascontiguousarray(
            np.stack([wgr[sA], wgr[sB]]).transpose(1, 2, 0, 3, 4)
        )                                                           # [KD,128,2,KD,128]
        biasT = np.concatenate([bcol[sA], bcol[sB]], axis=1)        # [128, 2*KD]

        # ---- popularity split: dense = top-RD referenced rows ----
        negs = ki[:, 1:]                                            # [1120, 16]
        cnt = np.bincount(negs.ravel(), minlength=R)
        order = np.argsort(-cnt, kind="stable")
        dense_rows = list(order[:RD].tolist())
        dense_set = np.zeros(R, bool)
        dense_set[dense_rows] = True
        tiles_rows = [
            np.unique(negs[t * 128 : (t + 1) * 128].ravel()) for t in range(NT)
        ]
        for _ in range(200):  # promotion w/ eviction (no-op on real data)
            over = None
            for t in range(NT):
                cold_t = tiles_rows[t][~dense_set[tiles_rows[t]]]
                if len(cold_t) > GT:
                    over = (t, cold_t)
                    break
            if over is None:
                break
            t, cold_t = over
            trows = negs[t * 128 : (t + 1) * 128].ravel()
            best = max(cold_t.tolist(), key=lambda r: int((trows == r).sum()))
            evict = min(
                (r for r in dense_rows if r != best), key=lambda r: int(cnt[r])
            )
            dense_rows[dense_rows.index(evict)] = best
            dense_set[evict] = False
            dense_set[best] = True
        else:
            raise RuntimeError("cold-slot overflow: could not balance tiles")
        dense_rows = np.asarray(dense_rows, np.int64)
        dcol = np.full(R, -1, np.int64)
        dcol[dense_rows] = np.arange(RD)

        cold_cols = np.full((NT, R), -1, np.int64)
        cold_pad = np.zeros((NT, GT), np.int64)
        for t in range(NT):
            cold_t = tiles_rows[t][~dense_set[tiles_rows[t]]]
            assert len(cold_t) <= GT
            cold_pad[t, : len(cold_t)] = cold_t
            cold_cols[t, cold_t] = np.arange(len(cold_t))

        encT = _to_partfirst(enc8[dense_rows].T)
        ecold = np.stack(
            [_to_partfirst(enc8[cold_pad[t]].T) for t in range(NT)]
        )                                                           # [NT,128,KD,GT]

        # negatives-only multiplicity mask over [dense ++ cold] columns
        p_idx = np.repeat(np.arange(PC), NEG)
        r_idx = negs.ravel()
        t_idx = p_idx // 128
        dc = dcol[r_idx]
        cc = cold_cols[t_idx, r_idx]
        col = np.where(dc >= 0, dc, RD + cc)
        assert ((dc >= 0) | (cc >= 0)).all()
        mm = np.zeros((PP, MW), np.float32)
        np.add.at(mm, (p_idx, col), 1.0)
        with np.errstate(divide="ignore"):
            cmv = np.where(mm > 0, np.log(np.maximum(mm, 1.0)), NEGINF).astype(
                np.float32
            )
        cmv[PC:, :] = NEGINF
        cmv[PC:, 0] = 0.0

        in_maps.append(
            {
                "ctxT": ctxT,
                "Wg": Wg,
                "biasT": biasT,
                "encT": encT,
                "ecold": ecold,
                "encTtgt": encTtgt,
                "cmask": cmv.astype(E4),
                "vmask": vmask,
                "ident": ident,
            }
        )
    return in_maps


def _install_ntff_hook():
    """Provide antenv.axon_hooks if the image lacks it, so trace=True can
    capture NTFF profiles through the injected libaxon_pjrt.so."""
    import sys
    import types
    import ctypes
    import contextlib
    import os

    try:
        from antenv.axon_hooks import get_axon_ntff_profile_hook  # noqa: F401

        return
    except ImportError:
        pass
    so_path = "/opt/axon/libaxon_pjrt.so"
    if not os.path.exists(so_path):
        return
    lib = ctypes.CDLL(so_path)
    if not hasattr(lib, "axon_start_nrt_profile"):
        return
    lib.axon_start_nrt_profile.argtypes = [
        ctypes.POINTER(ctypes.c_int64),
        ctypes.c_size_t,
    ]
    lib.axon_start_nrt_profile.restype = ctypes.c_int64
    lib.axon_stop_nrt_profile.argtypes = [ctypes.c_char_p]
    lib.axon_stop_nrt_profile.restype = ctypes.c_int64

    @contextlib.contextmanager
    def _hook(output_dir, device_ids):
        import jax

        jax.devices()
        if device_ids:
            ids = (ctypes.c_int64 * len(device_ids))(*device_ids)
            rc = lib.axon_start_nrt_profile(ids, len(device_ids))
        else:
            rc = lib.axon_start_nrt_profile(None, 0)
        if rc != 0:
            raise RuntimeError(f"axon_start_nrt_profile rc={rc}")
        try:
            yield
        finally:
            n = lib.axon_stop_nrt_profile(str(output_dir).encode())
            print(f"ntff profile: {n} file(s) written to {output_dir}")

    mod = types.ModuleType("antenv.axon_hooks")
    mod.get_axon_ntff_profile_hook = lambda: _hook
    mod.set_axon_ntff_profile_hook = lambda h: None
    sys.modules["antenv.axon_hooks"] = mod


def run(inputs, trace=False, **kwargs):
    """Run the SPMD kernel; returns (loss, correct, BassKernelResults)."""
    if trace:
        _install_ntff_hook()
    nc = _build()
    in_maps = _prep_in_maps(**inputs)
    res = run_bass_kernel_spmd(
        nc, in_maps, core_ids=list(range(N_CORES)), trace=trace, **kwargs
    )
    sums = np.stack([r["out"].reshape(2) for r in res.results])  # [8, 2]
    tot = sums.sum(axis=0, dtype=np.float64)
    loss = np.float32(tot[0] / P_TOTAL)
    correct = np.float32(tot[1] / P_TOTAL)
    return loss, correct, res


def kernel(**inputs):
    loss, correct, _ = run(inputs, trace=False)
    return loss, correct
